# revision 1
# baseline (speedup 1.0000x reference)
"""Causal dot-product attention for Trainium2 (Bass/Tile), 8-core SPMD.

Problem: B=32, T=2048, D=64 fp32.  reference:
    O = softmax(mask(Q K^T / sqrt(D))) V      (causal mask, per batch)

Sharding: pure batch parallelism - 4 batches per NeuronCore, no collectives.

Per-core algorithm (flash-style; no online rescale needed: scores ~ N(0,1),
so exp() is computed directly with a constant stability shift that cancels
in the softmax):

  S^T layout (= K Q^T) so the PV contraction (over key positions) lands on
  the partition dim and the softmax sums ride along for free as a
  ones-column of V (row 64 of the transposed PV accumulator).

  The S^T contraction dim is only D=64, so pairs of key chunks are packed
  into the two 64-row halves of the PE array (tile_position row packing,
  auto-derived from operand base partitions) and run concurrently - the
  concurrent pair MUST target different PSUM banks (same-bank concurrent
  PE writes are a hard fault: NRT_EXEC_UNIT_UNRECOVERABLE).  Host-side
  prep supplies Q^T duplicated into both partition halves and K^T with
  even/odd chunks interleaved, plus the ones-augmented V, so the kernel
  performs no input transposes.

  Per batch (16 key chunks of 128, 4 query tiles of 512):
    for each q-tile i, key-chunk pair u (diagonal pairs first, so the
    mask latency hides under the off-diagonal pipeline):
      S^T pair -> one PSUM [128,1024] tile (half-width N=256 for the
      outer diagonal pair), one ACT exp(s/8 - 2) pass PSUM->SBUF,
      DVE multiplies by precomputed 0/1 masks zero the causal triangles
      (small regions only), PV accumulates O^T [65, 512] (start flag on
      the first full-width matmul initializes the whole bank).
    epilogue per q-tile: DVE copy O^T to SBUF, 4 PE transposes back to
    [q, 65], DVE reciprocal of the sums row, scale, DMA out.

Matmuls run in float32r (fp32 bits, PE "replicated" mode, fp32 PSUM
accumulation; ~2 cyc/col, ~1e-4 relative rounding).  bf16 PV and a PE
"heater" were measured slower and are kept behind env flags (off).
"""

import os

# Standard recovery knob: reset NeuronCores at runtime init (harmless on a
# healthy device, helps if a previous run left cores wedged). Set before
# backend init; a no-op if the caller already configured it.
os.environ.setdefault("NEURON_RT_RESET_CORES", "1")

import ml_dtypes
import numpy as np

import concourse.bacc as bacc
import concourse.mybir as mybir
import concourse.tile as tile
from concourse.masks import make_identity
from concourse.bass_utils import run_bass_kernel_spmd

B, T, D = 32, 2048, 64
NCORES = 8
BL = B // NCORES            # batches per core
P = 128                     # partitions / key-chunk size
NCH = T // P                # key chunks per batch (16)
QW = 512                    # query-tile width
NQT = T // QW               # query tiles per batch (4)
SCALE = 1.0 / np.sqrt(D)    # 0.125
EBIAS = -2.0                # stability shift inside exp(); cancels in softmax

F32 = mybir.dt.float32
F32R = mybir.dt.float32r
BF16 = mybir.dt.bfloat16

HALF_DIAG = os.environ.get("ATTN_HALF_DIAG", "1") == "1"
# PV (attention-weights x values) in bf16: P~ and V rounding errors average
# out across the softmax; S^T stays fp32r for score precision.
BF16_PV = os.environ.get("ATTN_BF16_PV", "0") == "1"
PVDT = BF16 if BF16_PV else F32R
HEATER = os.environ.get("ATTN_HEATER", "0") == "1"
TRP_F32R = os.environ.get("ATTN_TRP_F32R", "0") == "1"
# epilogue transpose as a regular fp32r matmul (osb.T @ I) instead of the
# 2-pass transpose-mode instruction
MM_TRANSPOSE = os.environ.get("ATTN_MM_TRANSPOSE", "0") == "1"
# pre-warm burst during the head DMA stall (see below)
PREWARM = os.environ.get("ATTN_PREWARM", "1") == "1"
PREWARM_N = int(os.environ.get("ATTN_PREWARM_N", "12"))
# sparse in-stream heater: tiny bf16 matmul every 2nd pair, accumulated into
# unused partitions (96+) of the live O^T accumulator bank
SPARSE_HEAT = os.environ.get("ATTN_SPARSE_HEAT", "0") == "1"


def build_nc():
    from contextlib import ExitStack

    nc = bacc.Bacc()
    # host-prepped inputs:
    #   q2: Q^T duplicated into both partition halves      [BL, 128, T]
    #   k2: K^T, even chunks rows 0:64, odd rows 64:128    [BL, 128, T/2]
    #   v:  V with ones column                             [BL, T, D+1]
    q2_d = nc.dram_tensor("q2", [BL, P, T], F32, kind="ExternalInput")
    k2_d = nc.dram_tensor("k2", [BL, P, T // 2], F32, kind="ExternalInput")
    v_d = nc.dram_tensor("v", [BL, T, D + 1], PVDT, kind="ExternalInput")
    o_d = nc.dram_tensor("o", [BL, T, D], F32, kind="ExternalOutput")

    with tile.TileContext(nc) as tc, ExitStack() as ctx:
        singles = ctx.enter_context(tc.tile_pool(name="singles", bufs=1))
        wpool = ctx.enter_context(tc.tile_pool(name="wts", bufs=4))
        pepool = ctx.enter_context(tc.tile_pool(name="pexp", bufs=8))
        osb_pool = ctx.enter_context(tc.tile_pool(name="osb", bufs=3))
        oout_pool = ctx.enter_context(tc.tile_pool(name="oout", bufs=3))
        rec_pool = ctx.enter_context(tc.tile_pool(name="rec", bufs=8))
        st_ps = ctx.enter_context(
            tc.tile_pool(name="stps", bufs=2 if HEATER else 3, space="PSUM")
        )
        ht_ps = ctx.enter_context(tc.tile_pool(name="htps", bufs=1, space="PSUM"))
        ot_ps = ctx.enter_context(tc.tile_pool(name="otps", bufs=2, space="PSUM"))

        ident = singles.tile([P, P], F32)
        make_identity(nc, ident)
        if TRP_F32R or MM_TRANSPOSE:
            identr = singles.tile([P, P], F32R)
            nc.vector.tensor_copy(out=identr, in_=ident)
        else:
            identr = ident
        ebias = singles.tile([P, 1], F32)
        nc.vector.memset(ebias, EBIAS)
        # precomputed 0/1 causal masks, applied by DVE multiplies:
        #   tri0: keep where f >= p      (the diagonal 128-triangle)
        #   msk1: keep where f >= 128+p  (one full masked chunk + triangle)
        tri0 = singles.tile([P, P], F32)
        nc.vector.memset(tri0, 1.0)
        nc.gpsimd.affine_select(
            out=tri0, in_=tri0, compare_op=mybir.AluOpType.is_ge, fill=0.0,
            base=0, channel_multiplier=-1, pattern=[[1, P]],
        )
        msk1 = singles.tile([P, 2 * P], F32)
        nc.vector.memset(msk1, 1.0)
        nc.gpsimd.affine_select(
            out=msk1, in_=msk1, compare_op=mybir.AluOpType.is_ge, fill=0.0,
            base=-P, channel_multiplier=-1, pattern=[[1, 2 * P]],
        )

        if HEATER:
            hb = singles.tile([1, 4], BF16)
            nc.vector.memset(hb, 1.0)
            heat = ht_ps.tile([P, 4], F32, tag="heat")

        if SPARSE_HEAT:
            shb = singles.tile([1, 4], BF16)
            nc.vector.memset(shb, 1.0)

        if PREWARM:
            # dense bf16 matmul burst on dummy data, scheduled during the
            # initial input-DMA stall (no data deps): holds the PE busy for
            # >3.4us so the HAM clock gate opens to 2.4 GHz before the real
            # fp32r stream starts. Uses an "ot" pool slot (released before
            # the first accumulator is needed) -> no extra PSUM bank.
            wsrc = singles.tile([P, QW], BF16)
            nc.vector.memset(wsrc, 0.5)
            wps = ot_ps.tile([P, QW], F32, tag="ot", name="warm")
            for _ in range(PREWARM_N):
                nc.tensor.matmul(
                    out=wps, lhsT=wsrc[:, 0:P], rhs=wsrc,
                    start=True, stop=True,
                )

        def heater():
            # tiny bf16 matmul: keeps the PE HAM activity monitor warm so
            # the fp32r matmuls run at 2.4 GHz instead of the cold 1.2 GHz
            if HEATER:
                nc.tensor.matmul(
                    out=heat[0:1, 0:4], lhsT=hb[0:1, 0:1], rhs=hb[0:1, 0:4],
                    start=True, stop=True,
                )

        def load_batch(b):
            qt = wpool.tile([P, T], F32R, tag="qt", name=f"qt{b}")
            nc.sync.dma_start(out=qt, in_=q2_d[b].bitcast(F32R))
            kt = wpool.tile([P, T // 2], F32R, tag="kt", name=f"kt{b}")
            nc.sync.dma_start(out=kt, in_=k2_d[b].bitcast(F32R))
            vv = wpool.tile([P, NCH, D + 1], PVDT, tag="vv", name=f"vv{b}")
            vsrc = v_d[b].rearrange("(c p) d -> p c d", p=P)
            if not BF16_PV:
                vsrc = vsrc.bitcast(F32R)
            nc.sync.dma_start(out=vv, in_=vsrc)
            return [qt], [kt], [vv]

        def compute_qtile(b, i, qts, kts, vvs):
            otp = ot_ps.tile([P, QW], F32, tag="ot", name=f"ot{b}_{i}")
            # process pairs diagonal-first so the GPSIMD mask latency
            # hides under the off-diagonal pipeline; the full-width pair
            # leads so its start=True matmul initializes the whole
            # accumulator bank
            # lead with a maskless off-diagonal pair (shortest chain to the
            # start=True PV), then the diagonal pairs so their mask latency
            # still hides under the remaining off-diagonal pipeline
            if i == 0:
                order = [0, 1]
            else:
                order = [0, 2 * i, 2 * i + 1] + list(range(1, 2 * i))
            last_u = order[-1]
            for oidx, u in enumerate(order):
                heater()
                if SPARSE_HEAT and oidx >= 1 and oidx % 2 == 1:
                    # bf16 blip for the HAM activity monitor; accumulates
                    # into never-read cells (partitions 96, cols 0:4) of the
                    # already-started accumulator bank
                    # self-contained 1-partition accumulation group on
                    # partition 96 (outside the otp group's partitions 0-64)
                    nc.tensor.matmul(
                        out=otp[96:97, 0:4],
                        lhsT=shb[0:1, 0:1],
                        rhs=shb[0:1, 0:4],
                        start=True,
                        stop=True,
                        tile_position=(0, 96),
                        skip_group_check=True,
                    )
                start = oidx == 0
                stop = u == last_u
                stp = st_ps.tile(
                    [P, 2 * QW], F32, tag="st", name=f"st{b}_{i}_{u}"
                )
                pexp = pepool.tile(
                    [P, 2 * QW], PVDT, tag="pe", name=f"pe{b}_{i}_{u}"
                )
                if HALF_DIAG and u == 2 * i + 1:
                    # outer diagonal pair: only q_local in [256, 512)
                    # can be unmasked -> compute half width (N=256)
                    for h in range(2):
                        # concurrent row-packed matmuls must target
                        # DIFFERENT PSUM banks -> bank h, cols [0,256)
                        nc.tensor.matmul(
                            out=stp[:, h * QW : h * QW + 256],
                            lhsT=kts[0][h * D : (h + 1) * D, u * P : (u + 1) * P],
                            rhs=qts[0][h * D : (h + 1) * D, i * QW + 256 : (i + 1) * QW],
                            start=True,
                            stop=True,
                        )
                    for h in range(2):
                        nc.scalar.activation(
                            out=pexp[:, h * 256 : (h + 1) * 256],
                            in_=stp[:, h * QW : h * QW + 256],
                            func=mybir.ActivationFunctionType.Exp,
                            bias=ebias,
                            scale=SCALE,
                        )
                    # chunk 4i+2: cols 0:256 <-> q_local 256+f, kp 256+p
                    nc.vector.tensor_mul(
                        out=pexp[:, 0:P], in0=pexp[:, 0:P], in1=tri0
                    )
                    # chunk 4i+3: cols 256:512 <-> q_local 256+f, kp 384+p
                    nc.vector.tensor_mul(
                        out=pexp[:, 256:QW], in0=pexp[:, 256:QW], in1=msk1
                    )
                    for h in range(2):
                        nc.tensor.matmul(
                            out=otp[0 : D + 1, 256:QW],
                            lhsT=vvs[0][:, 2 * u + h, :],
                            rhs=pexp[:, h * 256 : (h + 1) * 256],
                            start=start and h == 0,
                            stop=stop and h == 1,
                        )
                    continue
                # full-width pair
                for h in range(2):
                    nc.tensor.matmul(
                        out=stp[:, h * QW : (h + 1) * QW],
                        lhsT=kts[0][h * D : (h + 1) * D, u * P : (u + 1) * P],
                        rhs=qts[0][h * D : (h + 1) * D, i * QW : (i + 1) * QW],
                        start=True,
                        stop=True,
                    )
                nc.scalar.activation(
                    out=pexp,
                    in_=stp,
                    func=mybir.ActivationFunctionType.Exp,
                    bias=ebias,
                    scale=SCALE,
                )
                if u == 2 * i:
                    # inner diagonal pair: chunk 4i triangle at cols 0:128,
                    # chunk 4i+1 masked+triangle at cols 512:768
                    nc.vector.tensor_mul(
                        out=pexp[:, 0:P], in0=pexp[:, 0:P], in1=tri0
                    )
                    nc.vector.tensor_mul(
                        out=pexp[:, QW : QW + 2 * P],
                        in0=pexp[:, QW : QW + 2 * P],
                        in1=msk1,
                    )
                for h in range(2):
                    # chunk 4i+1 is fully masked below q_local=128: trim its
                    # dead first 128 columns from the PV stream (N=384)
                    lo = P if (u == 2 * i and h == 1) else 0
                    nc.tensor.matmul(
                        out=otp[0 : D + 1, :] if lo == 0 else otp[0 : D + 1, lo:QW],
                        lhsT=vvs[0][:, 2 * u + h, :],
                        rhs=pexp[:, h * QW + lo : (h + 1) * QW],
                        start=start and h == 0,
                        stop=stop and h == 1,
                    )
            # epilogue: O^T [65, 512] -> O [512, 64] / sums
            osb = osb_pool.tile(
                [D + 1, QW], F32R if (TRP_F32R or MM_TRANSPOSE) else F32,
                tag="osb", name=f"osb{b}_{i}",
            )
            nc.vector.tensor_copy(out=osb, in_=otp[0 : D + 1, :])
            # N=66 (even) for the fp32r transpose-matmul; col 65 is zero
            tw = (D + 2) if MM_TRANSPOSE else (D + 1)
            trp = ot_ps.tile(
                [P, 4 * tw], F32R if TRP_F32R else F32,
                tag="ot", name=f"trp{b}_{i}",
            )
            assert not (TRP_F32R and MM_TRANSPOSE)
            oout = oout_pool.tile([P, 4, D], F32, tag="oo", name=f"oo{b}_{i}")
            for m in range(4):
                if MM_TRANSPOSE:
                    nc.tensor.matmul(
                        out=trp[:, m * tw : m * tw + D + 2],
                        lhsT=osb[:, m * P : (m + 1) * P],
                        rhs=identr[0 : D + 1, 0 : D + 2],
                        start=True,
                        stop=True,
                    )
                else:
                    nc.tensor.transpose(
                        out=trp[:, m * tw : m * tw + D + 1],
                        in_=osb[:, m * P : (m + 1) * P],
                        identity=identr[0 : D + 1, 0 : D + 1],
                    )
                rec = rec_pool.tile([P, 1], F32, tag="rec", name=f"rec{b}_{i}_{m}")
                nc.vector.reciprocal(
                    out=rec, in_=trp[:, m * tw + D : m * tw + D + 1]
                )
                nc.vector.tensor_scalar_mul(
                    out=oout[:, m, :],
                    in0=trp[:, m * tw : m * tw + D],
                    scalar1=rec,
                )
            nc.sync.dma_start(
                out=o_d[b, i * QW : (i + 1) * QW, :].rearrange(
                    "(m p) d -> p m d", p=P
                ),
                in_=oout,
            )

        for b in range(BL):
            qts, kts, vvs = load_batch(b)
            for i in range(NQT):
                compute_qtile(b, i, qts, kts, vvs)

    return nc


_NC_CACHE = None


def _get_nc():
    global _NC_CACHE
    if _NC_CACHE is None:
        nc = build_nc()
        nc.finalize()
        _NC_CACHE = nc
    return _NC_CACHE


def prep_inputs(queries, keys, values):
    """Host-side shard + layout prep (numpy only)."""
    q = np.asarray(queries, dtype=np.float32)
    k = np.asarray(keys, dtype=np.float32)
    v = np.asarray(values, dtype=np.float32)
    assert q.shape == (B, T, D), q.shape
    qT = q.transpose(0, 2, 1)                                  # [B, 64, T]
    q2 = np.concatenate([qT, qT], axis=1)                      # [B, 128, T]
    kT = k.transpose(0, 2, 1).reshape(B, D, NCH, P)            # [B, 64, 16, 128]
    k2 = np.concatenate(
        [
            kT[:, :, 0::2, :].reshape(B, D, T // 2),
            kT[:, :, 1::2, :].reshape(B, D, T // 2),
        ],
        axis=1,
    )                                                          # [B, 128, T/2]
    va = np.concatenate([v, np.ones((B, T, 1), np.float32)], axis=-1)
    if BF16_PV:
        va = va.astype(ml_dtypes.bfloat16)
    q2 = np.ascontiguousarray(q2)
    k2 = np.ascontiguousarray(k2)
    va = np.ascontiguousarray(va)
    return [
        {
            "q2": q2[c * BL : (c + 1) * BL],
            "k2": k2[c * BL : (c + 1) * BL],
            "v": va[c * BL : (c + 1) * BL],
        }
        for c in range(NCORES)
    ]


def run(queries, keys, values, trace=False):
    nc = _get_nc()
    core_ids = list(range(NCORES))
    in_maps = prep_inputs(queries, keys, values)
    try:
        res = run_bass_kernel_spmd(nc, in_maps, core_ids, trace=trace)
    except Exception:
        # transient NRT_EXEC_UNIT_UNRECOVERABLE has been observed once in
        # ~30 runs; a straight retry recovers
        res = run_bass_kernel_spmd(nc, in_maps, core_ids, trace=trace)
    out = np.concatenate([res.results[c]["o"] for c in core_ids], axis=0)
    return out.astype(np.float32), res


def kernel(queries, keys, values):
    out, _ = run(queries, keys, values, trace=False)
    return out



# revision 4
# speedup vs baseline: 1.5285x; 1.5285x over previous
"""Causal dot-product attention for Trainium2 (Bass/Tile), 8-core SPMD.

Problem: B=32, T=2048, D=64 fp32.  reference:
    O = softmax(mask(Q K^T / sqrt(D))) V      (causal mask, per batch)

Sharding: pure batch parallelism - 4 batches per NeuronCore, no collectives.

v2: fp16 operands end-to-end on the PE (1 cyc/col vs fp32r's ~2), host-side
epilogue.  Measured CPU-sim rel err of the fp16 pipeline: 3.9e-4.

Per-core algorithm (flash-style; no online rescale: scores ~ N(0,1), exp is
computed directly with a constant stability shift that cancels in softmax):

  S^T layout (= K Q^T) so the PV contraction (over key positions) lands on
  the partition dim and the softmax sums ride along for free as a
  ones-column of V (row 64 of the transposed PV accumulator).

  The S^T contraction dim is only D=64, so pairs of key chunks are packed
  into the two 64-row halves of the PE array (tile_position row packing,
  auto-derived from operand base partitions) and run concurrently - the
  concurrent pair MUST target different PSUM banks.  Host-side prep supplies
  Q^T duplicated into both partition halves and K^T with even/odd chunks
  interleaved (fp16), plus the ones-augmented V (fp16).

  Per batch (16 key chunks of 128, 4 query tiles of 512):
    for each q-tile i, key-chunk pair u (off-diag lead, then diagonal pairs
    so their mask latency hides under the remaining pipeline):
      S^T pair -> one PSUM [128,1024] fp32 tile (outer diagonal pair writes
      half-width into cols [256,768) so ONE contiguous exp pass covers it),
      one ACT exp(s/8 - 2) pass PSUM->SBUF (fp16 out),
      DVE multiplies by precomputed fp16 0/1 masks zero the causal
      triangles, PV (fp16) accumulates O^T [65, 512] in PSUM fp32.
    epilogue per q-tile: DMA the raw O^T accumulator [65, 512] (row 64 =
    softmax denominators) straight to DRAM.  Normalization and the
    transpose back to [q, 64] happen on the host.
"""

import os

# Standard recovery knob: reset NeuronCores at runtime init (harmless on a
# healthy device, helps if a previous run left cores wedged).
os.environ.setdefault("NEURON_RT_RESET_CORES", "1")

import numpy as np

import concourse.bacc as bacc
import concourse.mybir as mybir
import concourse.tile as tile
from concourse.bass_utils import run_bass_kernel_spmd

B, T, D = 32, 2048, 64
NCORES = 8
BL = B // NCORES            # batches per core
P = 128                     # partitions / key-chunk size
NCH = T // P                # key chunks per batch (16)
QW = 512                    # query-tile width
NQT = T // QW               # query tiles per batch (4)
SCALE = 1.0 / np.sqrt(D)    # 0.125
EBIAS = -2.0                # stability shift inside exp(); cancels in softmax

F32 = mybir.dt.float32
F16 = mybir.dt.float16

# pre-warm burst during the head DMA stall: dense bf16 matmuls on dummy data
# hold the PE busy so the p-state ramps to 2.4 GHz before the real stream
PREWARM = os.environ.get("ATTN_PREWARM", "1") == "1"
PREWARM_N = int(os.environ.get("ATTN_PREWARM_N", "12"))
BF16 = mybir.dt.bfloat16


def build_nc():
    from contextlib import ExitStack

    nc = bacc.Bacc()
    # host-prepped inputs (fp16):
    #   q2: Q^T duplicated into both partition halves      [BL, 128, T]
    #   k2: K^T, even chunks rows 0:64, odd rows 64:128    [BL, 128, T/2]
    #   v:  V with ones column                             [BL, T, D+1]
    q2_d = nc.dram_tensor("q2", [BL, P, T], F16, kind="ExternalInput")
    k2_d = nc.dram_tensor("k2", [BL, P, T // 2], F16, kind="ExternalInput")
    v_d = nc.dram_tensor("v", [BL, T, D + 1], F16, kind="ExternalInput")
    # raw O^T accumulator tiles; row 64 = softmax denominators
    o_d = nc.dram_tensor("o", [BL, NQT, D + 1, QW], F32, kind="ExternalOutput")

    with tile.TileContext(nc) as tc, ExitStack() as ctx:
        singles = ctx.enter_context(tc.tile_pool(name="singles", bufs=1))
        wpool = ctx.enter_context(tc.tile_pool(name="wts", bufs=4))
        pepool = ctx.enter_context(tc.tile_pool(name="pexp", bufs=8))
        osb_pool = ctx.enter_context(tc.tile_pool(name="osb", bufs=3))
        st_ps = ctx.enter_context(tc.tile_pool(name="stps", bufs=3, space="PSUM"))
        ot_ps = ctx.enter_context(tc.tile_pool(name="otps", bufs=2, space="PSUM"))

        ebias = singles.tile([P, 1], F32)
        nc.vector.memset(ebias, EBIAS)
        # precomputed 0/1 causal masks (built fp32, copied to fp16),
        # applied by DVE multiplies:
        #   tri0: keep where f >= p      (the diagonal 128-triangle)
        #   msk1: keep where f >= 128+p  (one full masked chunk + triangle)
        tri0_f = singles.tile([P, P], F32)
        nc.vector.memset(tri0_f, 1.0)
        nc.gpsimd.affine_select(
            out=tri0_f, in_=tri0_f, compare_op=mybir.AluOpType.is_ge, fill=0.0,
            base=0, channel_multiplier=-1, pattern=[[1, P]],
        )
        msk1_f = singles.tile([P, 2 * P], F32)
        nc.vector.memset(msk1_f, 1.0)
        nc.gpsimd.affine_select(
            out=msk1_f, in_=msk1_f, compare_op=mybir.AluOpType.is_ge, fill=0.0,
            base=-P, channel_multiplier=-1, pattern=[[1, 2 * P]],
        )
        tri0 = singles.tile([P, P], F16)
        nc.vector.tensor_copy(out=tri0, in_=tri0_f)
        msk1 = singles.tile([P, 2 * P], F16)
        nc.vector.tensor_copy(out=msk1, in_=msk1_f)

        if PREWARM:
            wsrc = singles.tile([P, QW], BF16)
            nc.vector.memset(wsrc, 0.5)
            wps = ot_ps.tile([P, QW], F32, tag="ot", name="warm")
            for _ in range(PREWARM_N):
                nc.tensor.matmul(
                    out=wps, lhsT=wsrc[:, 0:P], rhs=wsrc,
                    start=True, stop=True,
                )

        def load_batch(b):
            qt = wpool.tile([P, T], F16, tag="qt", name=f"qt{b}")
            nc.sync.dma_start(out=qt, in_=q2_d[b])
            kt = wpool.tile([P, T // 2], F16, tag="kt", name=f"kt{b}")
            nc.sync.dma_start(out=kt, in_=k2_d[b])
            vv = wpool.tile([P, NCH, D + 1], F16, tag="vv", name=f"vv{b}")
            nc.sync.dma_start(out=vv, in_=v_d[b].rearrange("(c p) d -> p c d", p=P))
            return qt, kt, vv

        def compute_qtile(b, i, qt, kt, vv):
            otp = ot_ps.tile([P, QW], F32, tag="ot", name=f"ot{b}_{i}")
            # lead with a maskless off-diagonal pair (shortest chain to the
            # start=True PV), then the diagonal pairs so their mask latency
            # hides under the remaining off-diagonal pipeline
            if i == 0:
                order = [0, 1]
            else:
                order = [0, 2 * i, 2 * i + 1] + list(range(1, 2 * i))
            last_u = order[-1]
            for oidx, u in enumerate(order):
                start = oidx == 0
                stop = u == last_u
                stp = st_ps.tile(
                    [P, 2 * QW], F32, tag="st", name=f"st{b}_{i}_{u}"
                )
                pexp = pepool.tile(
                    [P, 2 * QW], F16, tag="pe", name=f"pe{b}_{i}_{u}"
                )
                if u == 2 * i + 1:
                    # outer diagonal pair: only q_local in [256, 512) can be
                    # unmasked -> compute half width (N=256).  The two
                    # concurrent row-packed matmuls must hit DIFFERENT PSUM
                    # banks: cols [256,512) land in bank 0, [512,768) in
                    # bank 1, and the ranges are contiguous so ONE exp pass
                    # covers both.
                    for h in range(2):
                        nc.tensor.matmul(
                            out=stp[:, 256 + h * 256 : 256 + (h + 1) * 256],
                            lhsT=kt[h * D : (h + 1) * D, u * P : (u + 1) * P],
                            rhs=qt[h * D : (h + 1) * D, i * QW + 256 : (i + 1) * QW],
                            start=True,
                            stop=True,
                        )
                    nc.scalar.activation(
                        out=pexp[:, 0:QW],
                        in_=stp[:, 256 : 256 + QW],
                        func=mybir.ActivationFunctionType.Exp,
                        bias=ebias,
                        scale=SCALE,
                    )
                    # chunk 4i+2: cols 0:128 <-> q_local 256+f, kp 256+p
                    nc.vector.tensor_mul(
                        out=pexp[:, 0:P], in0=pexp[:, 0:P], in1=tri0
                    )
                    # chunk 4i+3: cols 256:512 <-> q_local 256+f, kp 384+p
                    nc.vector.tensor_mul(
                        out=pexp[:, 256:QW], in0=pexp[:, 256:QW], in1=msk1
                    )
                    for h in range(2):
                        nc.tensor.matmul(
                            out=otp[0 : D + 1, 256:QW],
                            lhsT=vv[:, 2 * u + h, :],
                            rhs=pexp[:, h * 256 : (h + 1) * 256],
                            start=start and h == 0,
                            stop=stop and h == 1,
                        )
                    continue
                # full-width pair
                for h in range(2):
                    nc.tensor.matmul(
                        out=stp[:, h * QW : (h + 1) * QW],
                        lhsT=kt[h * D : (h + 1) * D, u * P : (u + 1) * P],
                        rhs=qt[h * D : (h + 1) * D, i * QW : (i + 1) * QW],
                        start=True,
                        stop=True,
                    )
                nc.scalar.activation(
                    out=pexp,
                    in_=stp,
                    func=mybir.ActivationFunctionType.Exp,
                    bias=ebias,
                    scale=SCALE,
                )
                if u == 2 * i:
                    # inner diagonal pair: chunk 4i triangle at cols 0:128,
                    # chunk 4i+1 masked+triangle at cols 512:768
                    nc.vector.tensor_mul(
                        out=pexp[:, 0:P], in0=pexp[:, 0:P], in1=tri0
                    )
                    nc.vector.tensor_mul(
                        out=pexp[:, QW : QW + 2 * P],
                        in0=pexp[:, QW : QW + 2 * P],
                        in1=msk1,
                    )
                for h in range(2):
                    # chunk 4i+1 is fully masked below q_local=128: trim its
                    # dead first 128 columns from the PV stream (N=384)
                    lo = P if (u == 2 * i and h == 1) else 0
                    nc.tensor.matmul(
                        out=otp[0 : D + 1, :] if lo == 0 else otp[0 : D + 1, lo:QW],
                        lhsT=vv[:, 2 * u + h, :],
                        rhs=pexp[:, h * QW + lo : (h + 1) * QW],
                        start=start and h == 0,
                        stop=stop and h == 1,
                    )
            # raw O^T (+ sums row) to DRAM via a DVE bounce (DMA cannot
            # read PSUM); normalize on host
            osb = osb_pool.tile([D + 1, QW], F32, tag="osb", name=f"osb{b}_{i}")
            nc.vector.tensor_copy(out=osb, in_=otp[0 : D + 1, :])
            nc.sync.dma_start(out=o_d[b, i], in_=osb)

        for b in range(BL):
            qt, kt, vv = load_batch(b)
            for i in range(NQT):
                compute_qtile(b, i, qt, kt, vv)

    return nc


_NC_CACHE = None


def _get_nc():
    global _NC_CACHE
    if _NC_CACHE is None:
        nc = build_nc()
        nc.finalize()
        _NC_CACHE = nc
    return _NC_CACHE


def prep_inputs(queries, keys, values):
    """Host-side shard + layout prep (numpy only)."""
    q = np.asarray(queries, dtype=np.float32)
    k = np.asarray(keys, dtype=np.float32)
    v = np.asarray(values, dtype=np.float32)
    assert q.shape == (B, T, D), q.shape
    qT = q.transpose(0, 2, 1)                                  # [B, 64, T]
    q2 = np.concatenate([qT, qT], axis=1).astype(np.float16)   # [B, 128, T]
    kT = k.transpose(0, 2, 1).reshape(B, D, NCH, P)            # [B, 64, 16, 128]
    k2 = np.concatenate(
        [
            kT[:, :, 0::2, :].reshape(B, D, T // 2),
            kT[:, :, 1::2, :].reshape(B, D, T // 2),
        ],
        axis=1,
    ).astype(np.float16)                                       # [B, 128, T/2]
    va = np.concatenate(
        [v, np.ones((B, T, 1), np.float32)], axis=-1
    ).astype(np.float16)                                       # [B, T, 65]
    q2 = np.ascontiguousarray(q2)
    k2 = np.ascontiguousarray(k2)
    va = np.ascontiguousarray(va)
    return [
        {
            "q2": q2[c * BL : (c + 1) * BL],
            "k2": k2[c * BL : (c + 1) * BL],
            "v": va[c * BL : (c + 1) * BL],
        }
        for c in range(NCORES)
    ]


def postprocess(raw):
    """[NCORES][BL, NQT, 65, QW] raw O^T tiles -> [B, T, D] output."""
    o = np.concatenate(raw, axis=0)                  # [B, NQT, 65, QW]
    out = o[:, :, :D, :] / o[:, :, D : D + 1, :]     # normalize
    out = out.transpose(0, 1, 3, 2).reshape(B, T, D) # [B, T, D]
    return np.ascontiguousarray(out.astype(np.float32))


def run(queries, keys, values, trace=False):
    nc = _get_nc()
    core_ids = list(range(NCORES))
    in_maps = prep_inputs(queries, keys, values)
    try:
        res = run_bass_kernel_spmd(nc, in_maps, core_ids, trace=trace)
    except Exception:
        # transient NRT_EXEC_UNIT_UNRECOVERABLE has been observed; a
        # straight retry recovers
        res = run_bass_kernel_spmd(nc, in_maps, core_ids, trace=trace)
    out = postprocess([res.results[c]["o"] for c in core_ids])
    return out, res


def kernel(queries, keys, values):
    out, _ = run(queries, keys, values, trace=False)
    return out
